# revision 31
# baseline (speedup 1.0000x reference)
"""Trainium2 Bass kernel for DigitConvolutionalModel.

Computes: out = relu(conv2d_valid(x.reshape(B,28,28), w3x3)).reshape(B,676) @ W + b

Strategy (pure data parallel over 8 NeuronCores, 8192 images/core), bf16:
  - Host: pack x per core into 16 blocks of [120 part, 7 chunks, 512 batch]
    bf16, fully contiguous per block (one 860KB DMA per block). Chunk c =
    image rows 4c..4c+3. Even chunks store rows [0,1,2,3] at partitions
    0..111; odd chunks store rows [2,3] at partitions 0..55 and rows [0,1]
    at partitions 64..119 (so B-phase matmul pairs land on disjoint PE row
    strips and run concurrently).
  - Device per 512-image block (13 PE pass-equivalents of N=512):
      A-phase: 7 matmuls (stationary CAe/CAo [120,128], K=120) -> 7 PSUM
               banks, one per 4-output-row conv group.
      B-phase: 6 matmuls (stationary CB [56,104], K=56) issued as 3
               concurrent row-tiled pairs (tile_position (64,0)/(0,0)),
               closing groups 0..5.
      ReLU PSUM->SBUF bf16 (ACT: groups 0,2,4,6; DVE: 1,3,5).
      FC: 7 matmuls (W chunks [104,32], M=32) col-tiled over 4 strips
          (tile_position (0,32j)) -> 2 rounds into one PSUM bank;
          DVE copy -> SBUF; Sel matmul [128,10] reduces the 4 partials.
          FC for block j is emitted after conv of block j+1 (software
          pipelining) so the PE never waits on the ReLU copies.
      Bias-add via DVE -> outT store on the ACT HWDGE ring.
  - Host: gather per-core outT [10, 8192] and transpose into out[B, 10].
"""

import os

import numpy as np
import ml_dtypes

import concourse.bass as bass
import concourse.mybir as mybir
import concourse.tile as tile
from concourse import bacc
from concourse.bass import ts
from concourse.bass_utils import run_bass_kernel_spmd

BF16 = ml_dtypes.bfloat16
PRECISION = "bf16"

# Problem geometry (fixed by the task spec)
B_FULL = 65536
IMG = 28
KW = 3
OH = IMG - KW + 1          # 26
NPIX = IMG * IMG           # 784
NFEAT = OH * OH            # 676
NOUT = 10

N_CORES = 8
B_CORE = B_FULL // N_CORES  # 8192
NB = 512                    # images per block
N_BLOCKS = B_CORE // NB     # 16

G_ROWS = 4
G_FEAT = G_ROWS * OH        # 104
N_GROUPS = 7                # 6 groups of 4 out-rows + 1 group of 2 (52 feats)
CHUNK_ROWS = 4
XPART = 120                 # partitions used by the packed x layout
MPAD = 128                  # stationary columns padded to 128 (FWL)
FCM = 32                    # FC stationary columns (one 32-col strip)

WARM_MMS = 28               # HAM warm-up matmuls (N=512) before the first
                            # block: open the clock gate by ~6us and keep the
                            # PE busy until the first x block lands (~13us)


def _feat_count(t: int) -> int:
    return min(G_FEAT, NFEAT - G_FEAT * t)


def _part_base(c: int, r: int) -> int:
    """Partition base of image-row r (0..3) of chunk c in the packed layout."""
    if c % 2 == 0:
        return r * IMG
    return (r - 2) * IMG if r >= 2 else 64 + r * IMG


def build_conv_mats(conv_w: np.ndarray):
    """CA[pixel, feat] (within 4-row chunk) and CB[pixel, feat] (2-row head
    of the next chunk) express the 3x3 valid conv for one 4-out-row group."""
    w = np.asarray(conv_w, np.float32)
    CA = np.zeros((CHUNK_ROWS * IMG, G_FEAT), np.float32)
    CB = np.zeros((2 * IMG, G_FEAT), np.float32)
    for ol in range(G_ROWS):
        for oj in range(OH):
            m = ol * OH + oj
            for di in range(KW):
                for dj in range(KW):
                    r = ol + di
                    c = oj + dj
                    if r < CHUNK_ROWS:
                        CA[r * IMG + c, m] = w[di, dj]
                    else:
                        CB[(r - CHUNK_ROWS) * IMG + c, m] = w[di, dj]
    return CA, CB


def build_selector() -> np.ndarray:
    """S[32j + o, o] = 1: sums the 4 col-strip FC partials."""
    S = np.zeros((128, NOUT), np.float32)
    for j in range(4):
        for o in range(NOUT):
            S[32 * j + o, o] = 1.0
    return S


def build_program():
    f32 = mybir.dt.float32
    bf = mybir.dt.bfloat16

    nc = bacc.Bacc()
    xP = nc.declare_dram_parameter("xP", [N_BLOCKS, XPART, N_GROUPS, NB], bf,
                                   isOutput=False)
    cae_d = nc.declare_dram_parameter("CAe", [XPART, MPAD], bf, isOutput=False)
    cao_d = nc.declare_dram_parameter("CAo", [XPART, MPAD], bf, isOutput=False)
    cb_d = nc.declare_dram_parameter("CB2", [XPART, G_FEAT], bf, isOutput=False)
    wp_d = nc.declare_dram_parameter("Wp", [G_FEAT, N_GROUPS, FCM], bf,
                                     isOutput=False)
    sel_d = nc.declare_dram_parameter("Sel", [128, NOUT], bf, isOutput=False)
    bias_d = nc.declare_dram_parameter("bias", [NOUT, 1], f32, isOutput=False)
    outT = nc.declare_dram_parameter("outT", [NOUT, B_CORE], f32, isOutput=True)

    Relu = mybir.ActivationFunctionType.Relu
    Ident = mybir.ActivationFunctionType.Identity

    with tile.TileContext(nc) as tc:
        with (
            tc.tile_pool(name="const", bufs=1) as const,
            tc.tile_pool(name="x", bufs=6) as xpool,
            tc.tile_pool(name="feat", bufs=14) as fpool,
            tc.tile_pool(name="fcsb", bufs=2) as fcpool,
            tc.tile_pool(name="osb", bufs=3) as opool,
            tc.tile_pool(name="psc", bufs=7, space="PSUM") as psc,
            tc.tile_pool(name="pso", bufs=1, space="PSUM") as pso,
        ):
            # First x block issues before the constants: its 4us transfer
            # on the SP ring overlaps the const loads on the ACT ring.
            xt0 = xpool.tile([XPART, N_GROUPS, NB], bf, tag="x")
            nc.sync.dma_start(out=xt0[:, :, :], in_=xP[0, :, :, :])

            # Constants on the ACT HWDGE ring (x blocks use the SP ring).
            cae = const.tile([XPART, MPAD], bf)
            nc.scalar.dma_start(out=cae[:, :], in_=cae_d[:, :])
            cao = const.tile([XPART, MPAD], bf)
            nc.scalar.dma_start(out=cao[:, :], in_=cao_d[:, :])
            cb = const.tile([XPART, G_FEAT], bf)
            nc.scalar.dma_start(out=cb[:, :], in_=cb_d[:, :])
            wp = const.tile([G_FEAT, N_GROUPS, FCM], bf)
            nc.scalar.dma_start(out=wp[:, :, :], in_=wp_d[:, :, :])
            sel = const.tile([128, NOUT], bf)
            nc.scalar.dma_start(out=sel[:, :], in_=sel_d[:, :])
            bias_sb = const.tile([NOUT, 1], f32)
            nc.scalar.dma_start(out=bias_sb[:, :], in_=bias_d[:, :])

            # PE warm-up during the first x DMA (open the HAM clock gate).
            # The source tile is memset on GpSimd (no DMA dependency) so the
            # warm-up starts right after the engine preamble.
            wsrc = const.tile([MPAD, NB], bf)
            nc.gpsimd.memset(wsrc[:, :], 0)
            warm_ps = psc.tile([MPAD, NB], mybir.dt.float32, tag="convps")
            for _ in range(WARM_MMS):
                nc.tensor.matmul(
                    warm_ps[:, :], wsrc[:, :MPAD], wsrc[:, :],
                    start=True, stop=True,
                )

            state = {}

            def emit_conv(j, xc, a_order=(0, 2, 4, 6, 1, 3, 5)):
                # A-phase: even chunks (CAe) then odd chunks (CAo) so the
                # stationary operand changes only once. xc(t) -> chunk AP.
                pss = [None] * N_GROUPS
                for t in a_order:
                    ps = psc.tile([MPAD, NB], mybir.dt.float32, tag="convps")
                    ca = cae if t % 2 == 0 else cao
                    nc.tensor.matmul(
                        ps[:, :], ca[:, :], xc(t),
                        start=True, stop=(t == 6),
                    )
                    pss[t] = ps
                # B-phase: 3 row-tiled concurrent pairs.
                # B(t) reads chunk t+1 rows 0,1: odd chunk -> partitions
                # 64..119 (row strips 2,3); even chunk -> 0..55 (strips 0,1).
                for t in range(N_GROUPS - 1):
                    if t % 2 == 0:  # chunk t+1 odd: hi placement
                        nc.tensor.matmul(
                            pss[t][:G_FEAT, :], cb[64:120, :],
                            xc(t + 1)[64:120, :],
                            start=False, stop=True, tile_position=(64, 0),
                        )
                    else:           # chunk t+1 even: lo placement
                        nc.tensor.matmul(
                            pss[t][:G_FEAT, :], cb[0:56, :],
                            xc(t + 1)[0:56, :],
                            start=False, stop=True, tile_position=(0, 0),
                        )
                # ReLU PSUM->SBUF bf16 (ACT: 0,2,4,6; DVE: 1,3,5).
                feats = []
                for t in range(N_GROUPS):
                    nf = _feat_count(t)
                    ft = fpool.tile([nf, NB], bf, tag="feat")
                    if t % 2 == 0:
                        nc.scalar.activation(ft[:, :], pss[t][:nf, :], Relu)
                    else:
                        nc.vector.tensor_scalar_max(ft[:, :], pss[t][:nf, :], 0.0)
                    feats.append(ft)
                state[j] = feats

            def emit_fc(j):
                feats = state.pop(j)
                # 7 col-tiled matmuls into one PSUM bank: round 1 strips
                # 0..3 (each clears its strip), round 2 strips 0..2 accum.
                ops = pso.tile([128, NB], mybir.dt.float32, tag="outps")
                for t in range(N_GROUPS):
                    nf = _feat_count(t)
                    strip = 32 * (t % 4)
                    nc.tensor.matmul(
                        ops[strip:strip + FCM, :], wp[:nf, t, :],
                        feats[t][:, :],
                        start=(t < 4), stop=(t >= 3),
                        tile_position=(0, strip), skip_group_check=True,
                    )
                fcsb = fcpool.tile([128, NB], bf, tag="fcsb")
                nc.vector.tensor_copy(fcsb[:, :], ops[:, :])
                # Sel output reuses the fc-partial bank (already copied out).
                nc.tensor.matmul(
                    ops[:NOUT, :], sel[:, :], fcsb[:, :], start=True, stop=True,
                    skip_group_check=True,
                )
                osb = opool.tile([NOUT, NB], f32, tag="osb")
                nc.vector.tensor_scalar(
                    osb[:, :], ops[:NOUT, :], bias_sb[:, :], None,
                    op0=mybir.AluOpType.add,
                )
                nc.scalar.dma_start(out=outT[:, ts(j, NB)], in_=osb[:, :])

            for j in range(N_BLOCKS):
                if j == 0:
                    xt = xt0
                else:
                    xt = xpool.tile([XPART, N_GROUPS, NB], bf, tag="x")
                    nc.sync.dma_start(out=xt[:, :, :], in_=xP[j, :, :, :])
                emit_conv(j, lambda t, xt=xt: xt[:, t, :])
                if j >= 1:
                    emit_fc(j - 1)
            emit_fc(N_BLOCKS - 1)

    nc.finalize()
    return nc


def prepare_inputs(x, conv_w, W, b):
    CA, CB = build_conv_mats(conv_w)

    # Stationary conv matrices in the packed-partition layouts.
    CAe = np.zeros((XPART, MPAD), np.float32)
    CAe[: CHUNK_ROWS * IMG, :G_FEAT] = CA
    CAo = np.zeros((XPART, MPAD), np.float32)
    for r in range(CHUNK_ROWS):
        base = _part_base(1, r)
        CAo[base:base + IMG, :G_FEAT] = CA[r * IMG:(r + 1) * IMG, :]
    CB2 = np.zeros((XPART, G_FEAT), np.float32)
    CB2[0:2 * IMG, :] = CB       # lo placement (next chunk even)
    CB2[64:64 + 2 * IMG, :] = CB  # hi placement (next chunk odd)

    Wf = np.asarray(W, np.float32)
    Wp = np.zeros((G_FEAT, N_GROUPS, FCM), np.float32)
    for t in range(N_GROUPS):
        nf = _feat_count(t)
        Wp[:nf, t, :NOUT] = Wf[G_FEAT * t: G_FEAT * t + nf, :]
    Sel = build_selector()
    bias = np.asarray(b, np.float32).reshape(NOUT, 1)

    CAe, CAo, CB2, Wp, Sel = (a.astype(BF16) for a in (CAe, CAo, CB2, Wp, Sel))

    # Pack x: [B, 784] -> per core [N_BLOCKS, 120, 7, NB] bf16.
    xbf = np.asarray(x, np.float32).astype(BF16)
    # [core, block, b, row, col] view of the batch-major input
    xv = xbf.reshape(N_CORES, N_BLOCKS, NB, IMG, IMG)
    in_maps = []
    for core in range(N_CORES):
        xp = np.zeros((N_BLOCKS, XPART, N_GROUPS, NB), BF16)
        for c in range(N_GROUPS):
            for r in range(CHUNK_ROWS):
                base = _part_base(c, r)
                # [block, col, b] <- [block, b, col]
                xp[:, base:base + IMG, c, :] = (
                    xv[core, :, :, 4 * c + r, :].transpose(0, 2, 1)
                )
        in_maps.append(
            {
                "xP": xp,
                "CAe": CAe,
                "CAo": CAo,
                "CB2": CB2,
                "Wp": Wp,
                "Sel": Sel,
                "bias": bias,
            }
        )
    return in_maps


def _enable_ldw_opt():
    """Let walrus dedup repeated LDWEIGHTS so same-stationary matmul runs
    pipeline back-to-back instead of paying an LDW + drain per matmul."""
    import concourse.bass_utils as bu

    if getattr(bu, "_ldw_opt_patched", False):
        return
    orig = bu.run_command

    def patched(argv, **kw):
        argv = [
            "--enable-ldw-opt=true" if a == "--enable-ldw-opt=false" else a
            for a in argv
        ]
        return orig(argv, **kw)

    bu.run_command = patched
    bu._ldw_opt_patched = True


def run(x, conv_w, W, b, trace=False, **spmd_kwargs):
    if os.environ.get("KERNEL_LDW_OPT") == "1":
        _enable_ldw_opt()
    in_maps = prepare_inputs(x, conv_w, W, b)
    nc = build_program()
    res = run_bass_kernel_spmd(
        nc, in_maps, list(range(N_CORES)), trace=trace, **spmd_kwargs
    )
    out = np.empty((B_FULL, NOUT), np.float32)
    for c in range(N_CORES):
        out[c * B_CORE:(c + 1) * B_CORE, :] = res.results[c]["outT"].T
    return out, res


def kernel(x, conv_w, W, b):
    out, _ = run(x, conv_w, W, b, trace=False)
    return out
